# revision 4
# baseline (speedup 1.0000x reference)
"""Trainium2 Bass kernel for a single-layer transformer encoder.

Model: B=2, N=2048, D=1024, H=16, DFF=4096 (pre-computed QKV attention +
residual/LN + GELU FFN + residual/LN).

Sharding (zero-collective): 2 batches x 4-way sequence split. Core c owns
the 512 query tokens q=c%4 of batch b=c//4 and recomputes K/V for its whole
batch locally. Host-side roll puts each core's own tokens at columns 0:NT.
(Collectives were measured and rejected: a 4-rank intra-chip AllGather of
the 2MB K/V shard costs ~87us of un-hidable latency, more than the ~80us
of PE time the gather would save; and SPMD readback cannot skip the
self-block, so own-block overlap is impossible.)

Structure: the K/V recompute (109us of PE work) is interleaved into the
attention loop. Attention iterates key-chunk groups (4 groups of 512 keys)
outer, head-pairs inner; each group's K/V projection (PE) overlaps softmax
exp (ScalarE). The phase is PE-bound at the matmul stream rate (~214ns
per 512-col matmul, measured). K/V blocks stream through 2-deep SBUF
pools (a block dies when its group's AV matmuls finish) so no
full-sequence K/V is resident; attention output accumulates across groups
in SBUF (osum, fp32) via VectorE adds because PSUM's 8 banks cannot hold
16 live accumulators (psacc 2 + pspt 2x2 + psout 2 banks = 8 exactly).

Softmax runs on transposed scores PT[j, i] = exp(scale * k_j . q_i); the
denominators come from a ones-column appended to V (row 64 of the 65-row
attention-output accumulation). LayerNorm reductions over the feature
(partition) dim are ones-vector matmuls on the PE; per-token mean/rstd are
broadcast back across partitions with rank-1 fp32 matmuls (exact).

Precision: attention path bf16 operands / fp32 accumulation; the FFN runs
fp8e4m3 DoubleRow (2 k-chunks per matmul, ~2x PE throughput): w1/w2 are
pre-scaled by 256 host-side to center U(+-1/32) weights in e4m3's normal
range, and the descale folds free into the gelu activation scale and the
FFN-2 bias-apply. End-to-end rel err ~1.0e-2 (gate 2e-2); all-bf16 FFN
measures ~1.5x slower at rel err 3.4e-3 if margin is ever needed.

DMA: per-core DRAM->SBUF sustains ~360 GB/s (measured), so everything is
just-in-time: the sync queue carries the critical path (wq, xT own block,
wk, wv, rest of xT, wo reusing wk's slot = 12 MB) in first-use order; the
8 MB of fp8 FFN weights stream behind on the gpsimd queue into shallow
rotating slots whose recycling paces the stream against FFN consumption.

SBUF slot aliasing: QT -> z1 -> z2 share one 16 KB slot ("qz");
outT -> xln1 ("ox"); osum -> xln1f ("os"); xT -> hT ("xh"); wk -> wo
("kw"); per-group kt/vp/pt slots rotate in place.
"""

import os
import sys

for _p in ("/opt/trn_rl_repo", "/root/.axon_site", "/root/.axon_site/_ro/trn_rl_repo"):
    if os.path.isdir(_p) and _p not in sys.path:
        sys.path.append(_p)

import numpy as np

import concourse.bacc as bacc
import concourse.mybir as mybir
import concourse.tile as tile
from concourse.bass_utils import run_bass_kernel_spmd

P = 128
B, NSEQ, D, H, DFF = 2, 2048, 1024, 16, 4096
DH = D // H                     # 64
NT = 512                        # query tokens per core
DM = D // P                     # 8 feature chunks
JC = NSEQ // P                  # 16 key-token chunks
G = 4                           # key groups (512 keys each)
JG = JC // G                    # 4 jc per group
FC = DFF // P                   # 32 FFN feature chunks
HPAIRS = H // 2                 # 8
SCALE = DH ** -0.5
EPS = 1e-5

F32 = mybir.dt.float32
BF16 = mybir.dt.bfloat16
F8 = mybir.dt.float8e4
F8_SCALE = 256.0
AF = mybir.ActivationFunctionType

_NC_CACHE = None


def _rearr(ap):
    """DRAM [D_like, T] -> [p, chunk, T] view with chunk-major features."""
    return ap.rearrange("(c p) t -> p c t", p=P)


def _build_nc(reps=1, phases=("attn", "ffn")):
    nc = bacc.Bacc("TRN2", target_bir_lowering=False, debug=False)

    xT = nc.dram_tensor("xT", [D, NSEQ], BF16, kind="ExternalInput")
    # weights arrive pre-tiled: [out_chunk, partition, in_chunk, out_cols]
    w_q = nc.dram_tensor("w_q", [DM, P, DM, P], BF16, kind="ExternalInput")
    w_k = nc.dram_tensor("w_k", [DM, P, DM, P], BF16, kind="ExternalInput")
    w_v = nc.dram_tensor("w_v", [2, P, DM, 512], BF16, kind="ExternalInput")
    w_out = nc.dram_tensor("w_out", [DM, P, DM, P], BF16, kind="ExternalInput")
    w1 = nc.dram_tensor("w1", [DFF // 512, P, DM, 512], F8,
                        kind="ExternalInput")
    w2 = nc.dram_tensor("w2", [DM, P, FC, P], F8, kind="ExternalInput")
    b1 = nc.dram_tensor("b1", [DFF], F32, kind="ExternalInput")
    b2 = nc.dram_tensor("b2", [D], F32, kind="ExternalInput")
    ln1_w = nc.dram_tensor("ln1_w", [D], F32, kind="ExternalInput")
    ln1_b = nc.dram_tensor("ln1_b", [D], F32, kind="ExternalInput")
    ln2_w = nc.dram_tensor("ln2_w", [D], F32, kind="ExternalInput")
    ln2_b = nc.dram_tensor("ln2_b", [D], F32, kind="ExternalInput")
    yT = nc.dram_tensor("yT", [D, NT], F32, kind="ExternalOutput")

    with tile.TileContext(nc) as tc, \
         nc.allow_low_precision(reason="bf16 matmul operands; fp32 accum"):
        for _ in range(reps):
            _emit(nc, tc, xT, w_q, w_k, w_v, w_out, w1, w2, b1, b2,
                  ln1_w, ln1_b, ln2_w, ln2_b, yT, phases=phases)
    nc.compile()
    return nc


def _emit(nc, tc, xT_d, w_q, w_k, w_v, w_out, w1, w2, b1, b2,
          ln1_w, ln1_b, ln2_w, ln2_b, yT_d, phases=("attn", "ffn")):
    # ---------------- whole-kernel pools ----------------
    with tc.tile_pool(name="const", bufs=1) as pc, \
         tc.tile_pool(name="pers", bufs=1) as pers, \
         tc.tile_pool(name="scratch", bufs=3) as sq_pool, \
         tc.tile_pool(name="vecs", bufs=4) as vec_pool, \
         tc.tile_pool(name="psacc", bufs=2, space="PSUM") as psacc, \
         tc.tile_pool(name="pspt", bufs=2, space="PSUM") as pspt, \
         tc.tile_pool(name="psout", bufs=1, space="PSUM") as psout:

        # ---------------- constants ----------------
        ones_f32 = pc.tile([P, 2 * P], F32)
        nc.vector.memset(ones_f32[:], 1.0)
        ones_col = pc.tile([P, 1], BF16)          # lhsT for partition-sums
        nc.vector.tensor_copy(ones_col[:], ones_f32[:, 0:1])
        ones_row = pc.tile([1, P], F32)           # lhsT for exact broadcasts
        nc.vector.tensor_copy(ones_row[:], ones_f32[0:1, 0:P])
        eps_sb = pc.tile([1, 1], F32)
        nc.vector.memset(eps_sb[:], EPS)
        b1_sb = pc.tile([P, FC], F32)
        nc.sync.dma_start(b1_sb[:], b1.ap().rearrange("(c p) -> p c", p=P))
        b2_sb = pc.tile([P, DM], F32)
        nc.sync.dma_start(b2_sb[:], b2.ap().rearrange("(c p) -> p c", p=P))
        lnw1_sb = pc.tile([P, DM], F32)
        nc.sync.dma_start(lnw1_sb[:], ln1_w.ap().rearrange("(c p) -> p c", p=P))
        lnb1_sb = pc.tile([P, DM], F32)
        nc.sync.dma_start(lnb1_sb[:], ln1_b.ap().rearrange("(c p) -> p c", p=P))
        lnw2_sb = pc.tile([P, DM], F32)
        nc.sync.dma_start(lnw2_sb[:], ln2_w.ap().rearrange("(c p) -> p c", p=P))
        lnb2_sb = pc.tile([P, DM], F32)
        nc.sync.dma_start(lnb2_sb[:], ln2_b.ap().rearrange("(c p) -> p c", p=P))

        # persistent activations. Slot sharing: QT dies at the last scores
        # matmul, z1 at LN1, z2 born in FFN-2 ("qz"); outT dies at out-proj,
        # xln1 born in LN1 ("ox").
        QT = pers.tile([P, DM, NT], BF16, tag="qz")
        outT = pers.tile([P, DM, NT], BF16, tag="ox")
        osum = pers.tile([65, HPAIRS, 2 * NT], F32, tag="os")

        def ln_apply(z_tile, writes):
            """LayerNorm over features of z_tile [P, DM, NT] (fp32).
            writes(k, src_f32_ap) stores chunk k."""
            s1 = psacc.tile([1, NT], F32, tag="acc")
            s2 = psacc.tile([1, NT], F32, tag="acc")
            for k in range(DM):
                zb = sq_pool.tile([P, NT], BF16, tag="sq")
                nc.vector.tensor_copy(zb[:], z_tile[:, k, :])
                nc.tensor.matmul(s1[:], ones_col[:], zb[:],
                                 start=(k == 0), stop=(k == DM - 1))
                sq = sq_pool.tile([P, NT], BF16, tag="sq")
                nc.vector.tensor_mul(sq[:], zb[:], zb[:])
                nc.tensor.matmul(s2[:], ones_col[:], sq[:],
                                 start=(k == 0), stop=(k == DM - 1))
            mu = vec_pool.tile([1, NT], F32, tag="v")
            nc.vector.tensor_scalar_mul(mu[:], s1[:], 1.0 / D)
            var = vec_pool.tile([1, NT], F32, tag="v")
            nc.vector.tensor_scalar_mul(var[:], s2[:], 1.0 / D)
            musq = vec_pool.tile([1, NT], F32, tag="v")
            nc.vector.tensor_mul(musq[:], mu[:], mu[:])
            nc.vector.tensor_sub(var[:], var[:], musq[:])
            nc.scalar.activation(var[:], var[:], AF.Sqrt, bias=eps_sb[:])
            rec = vec_pool.tile([1, NT], F32, tag="v")
            nc.vector.reciprocal(rec[:], var[:])
            murf = vec_pool.tile([1, NT], F32, tag="v")
            nc.vector.tensor_mul(murf[:], mu[:], rec[:])
            R = psacc.tile([P, NT], F32, tag="acc")
            nc.tensor.matmul(R[:], ones_row[:], rec[:], start=True, stop=True)
            MR = psacc.tile([P, NT], F32, tag="acc")
            nc.tensor.matmul(MR[:], ones_row[:], murf[:], start=True, stop=True)
            for k in range(DM):
                t = sq_pool.tile([P, NT], F32, tag="sq")
                nc.vector.tensor_mul(t[:], z_tile[:, k, :], R[:])
                nc.vector.tensor_sub(t[:], t[:], MR[:])
                writes(k, t)

        with tc.tile_pool(name="wkvo", bufs=1) as pw, \
             tc.tile_pool(name="xpool", bufs=1) as px:
            xT = px.tile([P, DM, NSEQ], BF16, tag="xh")
            xTs = _rearr(xT_d.ap())
            wk_all = pw.tile([P, DM, DM, P], BF16, tag="kw")
            wv_all = pw.tile([P, 2, DM, 512], BF16)

            # -------- critical-path DMAs (sync queue, first-use order) ---
            with tc.tile_pool(name="wqp", bufs=1) as pwq:
                wq_all = pwq.tile([P, DM, DM, P], BF16)
                for qf in range(DM):
                    nc.sync.dma_start(wq_all[:, qf], w_q.ap()[qf])
                for k in range(DM):
                    nc.sync.dma_start(xT[:, k, 0:NT], xTs[:, k, 0:NT])
                for kf in range(DM):
                    nc.sync.dma_start(wk_all[:, kf], w_k.ap()[kf])
                for dvc in range(2):
                    nc.sync.dma_start(wv_all[:, dvc], w_v.ap()[dvc])
                for k in range(DM):
                    nc.sync.dma_start(xT[:, k, NT:NSEQ], xTs[:, k, NT:NSEQ])

                # -------- Q projection (own tokens) ----------------------
                if "attn" in phases:
                    for qf in range(DM):
                        acc = psacc.tile([P, NT], F32, tag="acc")
                        for k in range(DM):
                            nc.tensor.matmul(acc[:], wq_all[:, qf, k, :],
                                             xT[:, k, 0:NT],
                                             start=(k == 0), stop=(k == DM - 1))
                        nc.vector.tensor_copy(QT[:, qf, :], acc[:])
            # wqp closed: w1/w2 slots below reuse its bytes

            # -------- FFN weight streaming (gpsimd queue, shallow) -------
            w1ts, w2ts = [], []
            with tc.tile_pool(name="w1p", bufs=4) as w1_pool, \
                 tc.tile_pool(name="w2p", bufs=3) as w2_pool, \
                 tc.tile_pool(name="ktb", bufs=2) as kt_pool, \
                 tc.tile_pool(name="vpb", bufs=2) as vp_pool, \
                 tc.tile_pool(name="pt", bufs=3) as pt_pool:
                if "ffn" in phases:
                    for fg in range(DFF // 512):
                        w1t = w1_pool.tile([P, DM, 512], F8, tag="w1")
                        nc.gpsimd.dma_start(w1t[:], w1.ap()[fg])
                        w1ts.append(w1t)
                    for ef in range(DM):
                        w2t = w2_pool.tile([P, FC, P], F8, tag="w2")
                        nc.gpsimd.dma_start(w2t[:], w2.ap()[ef])
                        w2ts.append(w2t)

                if "attn" not in phases:      # timing-bisect stub
                    for k in range(DM):
                        nc.vector.tensor_copy(outT[:, k, :], xT[:, k, 0:NT])

                # -------- attention with interleaved K/V recompute -------
                if "attn" in phases:
                    for g in range(G):
                        jlo = g * JG
                        tlo, thi = jlo * P, (jlo + JG) * P
                        # K projection for this 512-key block
                        ktb = kt_pool.tile([P, DM, 512], BF16, tag="kt")
                        for kf in range(DM):
                            acc = psacc.tile([P, 512], F32, tag="acc")
                            for k in range(DM):
                                nc.tensor.matmul(
                                    acc[:], wk_all[:, kf, k, :],
                                    xT[:, k, tlo:thi],
                                    start=(k == 0), stop=(k == DM - 1))
                            nc.vector.tensor_copy(ktb[:, kf, :], acc[:])
                        # V projection (token-major) for this block
                        vpb = vp_pool.tile([P, JG, H * 65], BF16, tag="vp")
                        vp_h = vpb.rearrange("p j (h e) -> p j h e", e=65)
                        nc.vector.tensor_copy(
                            vp_h[:, :, :, 64:65],
                            ones_f32.rearrange("p (a b c) -> p a b c",
                                               a=JG, c=1)[:, :, 0:H, :])
                        for dvc in range(2):
                            for j in range(JG):
                                acc = psacc.tile([P, 512], F32, tag="acc")
                                for k in range(DM):
                                    nc.tensor.matmul(
                                        acc[:],
                                        xT[:, k, tlo + j * P:tlo + (j + 1) * P],
                                        wv_all[:, dvc, k, :],
                                        start=(k == 0), stop=(k == DM - 1))
                                nc.vector.tensor_copy(
                                    vp_h[:, j, dvc * 8:(dvc + 1) * 8, 0:64],
                                    acc[:].rearrange("p (h e) -> p h e", e=64))
                        # attention over this key block, all head-pairs
                        for hp in range(HPAIRS):
                            oacc = psout.tile([65, 2 * NT], F32, tag="o")
                            for j in range(JG):
                                pt_ps = pspt.tile([P, 2 * NT], F32, tag="pt")
                                for i in range(2):
                                    rows = slice(64 * i, 64 * i + 64)
                                    nc.tensor.matmul(
                                        pt_ps[:, i * NT:(i + 1) * NT],
                                        ktb[rows, hp, j * P:(j + 1) * P],
                                        QT[rows, hp, :],
                                        start=True, stop=True)
                                pt_sb = pt_pool.tile([P, 2 * NT], BF16,
                                                     tag="ptsb")
                                nc.scalar.activation(pt_sb[:], pt_ps[:],
                                                     AF.Exp, scale=SCALE)
                                for i in range(2):
                                    h = 2 * hp + i
                                    nc.tensor.matmul(
                                        oacc[:, i * NT:(i + 1) * NT],
                                        vpb[:, j, h * 65:(h + 1) * 65],
                                        pt_sb[:, i * NT:(i + 1) * NT],
                                        start=(j == 0), stop=(j == JG - 1))
                            if g == 0:
                                nc.vector.tensor_copy(osum[:, hp, :], oacc[:])
                            else:
                                nc.vector.tensor_add(osum[:, hp, :],
                                                     osum[:, hp, :], oacc[:])

                    # -------- normalize -> outT --------------------------
                    for hp in range(HPAIRS):
                        for i in range(2):
                            rec = vec_pool.tile([1, NT], F32, tag="v")
                            nc.vector.reciprocal(
                                rec[:], osum[64:65, hp, i * NT:(i + 1) * NT])
                            bc = psacc.tile([P, NT], F32, tag="acc")
                            nc.tensor.matmul(bc[0:64, :], ones_row[:, 0:64],
                                             rec[:], start=True, stop=True)
                            bc_sb = sq_pool.tile([P, NT], F32, tag="sq")
                            nc.vector.tensor_copy(bc_sb[0:64, :], bc[0:64, :])
                            nc.vector.tensor_mul(
                                outT[64 * i:64 * i + 64, hp, :],
                                osum[0:64, hp, i * NT:(i + 1) * NT],
                                bc_sb[0:64, :])

                # -------- output projection + residual 1 -----------------
                # wo reuses wk's slot (wk dies at the last K projection)
                wo_all = pw.tile([P, DM, DM, P], BF16, tag="kw")
                for ef in range(DM):
                    nc.sync.dma_start(wo_all[:, ef], w_out.ap()[ef])
                z1 = pers.tile([P, DM, NT], F32, tag="qz")  # reuses QT slot
                for ef in range(DM):
                    acc = psacc.tile([P, NT], F32, tag="acc")
                    for k in range(DM):
                        nc.tensor.matmul(acc[:], wo_all[:, ef, k, :],
                                         outT[:, k, :],
                                         start=(k == 0), stop=(k == DM - 1))
                    nc.vector.tensor_add(z1[:, ef, :], acc[:],
                                         xT[:, ef, 0:NT])

                # -------- LN1 --------------------------------------------
                xln1 = pers.tile([P, DM, NT], F8, tag="ox")   # reuses outT
                xln1f = pers.tile([P, DM, NT], F32, tag="os")  # reuses osum

                def write_xln1(k, t):
                    nc.vector.tensor_scalar(xln1f[:, k, :], t[:],
                                            lnw1_sb[:, k:k + 1],
                                            lnb1_sb[:, k:k + 1],
                                            op0=mybir.AluOpType.mult,
                                            op1=mybir.AluOpType.add)
                    nc.vector.tensor_copy(xln1[:, k, :], xln1f[:, k, :])
                ln_apply(z1, write_xln1)

                z2 = pers.tile([P, DM, NT], F32, tag="qz")  # reuses z1 slot

                if "ffn" not in phases:   # timing-bisect stub: LN2 input
                    for k in range(DM):
                        nc.vector.tensor_copy(z2[:, k, :], xln1f[:, k, :])

                # -------- FFN --------------------------------------------
                if "ffn" in phases:
                    hT = px.tile([P, FC, NT], F8, tag="xh")   # reuses xT slot
                    for fg in range(DFF // 512):
                        w1t = w1ts[fg]
                        for f4 in range(4):
                            f = fg * 4 + f4
                            acc = psacc.tile([P, NT], F32, tag="acc")
                            for k in range(0, DM, 2):
                                nc.tensor.matmul(
                                    acc[:],
                                    w1t[:, k:k + 2, f4 * P:(f4 + 1) * P],
                                    xln1[:, k:k + 2, :],
                                    start=(k == 0), stop=(k == DM - 2),
                                    perf_mode=mybir.MatmulPerfMode.DoubleRow)
                            nc.scalar.activation(hT[:, f, :], acc[:], AF.Gelu,
                                                 bias=b1_sb[:, f:f + 1],
                                                 scale=1.0 / F8_SCALE)

                    for ef in range(DM):
                        w2t = w2ts[ef]
                        acc = psacc.tile([P, NT], F32, tag="acc")
                        for k in range(0, FC, 2):
                            nc.tensor.matmul(acc[:], w2t[:, k:k + 2, :],
                                             hT[:, k:k + 2, :],
                                             start=(k == 0),
                                             stop=(k == FC - 2),
                                             perf_mode=mybir.MatmulPerfMode.DoubleRow)
                        t = sq_pool.tile([P, NT], F32, tag="sq")
                        nc.vector.tensor_scalar(t[:], acc[:], 1.0 / F8_SCALE,
                                                b2_sb[:, ef:ef + 1],
                                                op0=mybir.AluOpType.mult,
                                                op1=mybir.AluOpType.add)
                        nc.vector.tensor_add(z2[:, ef, :], t[:],
                                             xln1f[:, ef, :])

        # -------- LN2 -> output ------------------------------------------
        with tc.tile_pool(name="outstage", bufs=2) as out_pool:
            yT_r = _rearr(yT_d.ap())

            def write_out(k, t):
                o = out_pool.tile([P, NT], F32)
                nc.vector.tensor_scalar(o[:], t[:],
                                        lnw2_sb[:, k:k + 1],
                                        lnb2_sb[:, k:k + 1],
                                        op0=mybir.AluOpType.mult,
                                        op1=mybir.AluOpType.add)
                nc.sync.dma_start(yT_r[:, k, :], o[:])
            ln_apply(z2, write_out)


def _get_nc():
    global _NC_CACHE
    if _NC_CACHE is None:
        _NC_CACHE = _build_nc()
    return _NC_CACHE


def _tile_w(W, out_cols, dtype=None, scale=1.0):
    """[Din, Dout] f32 -> [Dout//out_cols, 128, Din//128, out_cols]
    so each output-chunk's weights are one contiguous DMA slab."""
    import ml_dtypes
    if dtype is None:
        dtype = ml_dtypes.bfloat16
    Din, Dout = W.shape
    t = (W * scale).astype(dtype).reshape(Din // P, P,
                                          Dout // out_cols, out_cols)
    return np.ascontiguousarray(t.transpose(2, 1, 0, 3))


def make_in_maps(x, w_qkv, w_out, ln1_w, ln1_b, w1, b1, w2, b2,
                 ln2_w, ln2_b):
    import ml_dtypes
    bf = ml_dtypes.bfloat16
    x = np.ascontiguousarray(np.asarray(x, dtype=np.float32))
    w_qkv = np.asarray(w_qkv, np.float32)
    shared = {
        "w_q": _tile_w(w_qkv[:, 0:D], P),
        "w_k": _tile_w(w_qkv[:, D:2 * D], P),
        "w_v": _tile_w(w_qkv[:, 2 * D:3 * D], 512),
        "w_out": _tile_w(np.asarray(w_out, np.float32), P),
        "w1": _tile_w(np.asarray(w1, np.float32), 512,
                      ml_dtypes.float8_e4m3, F8_SCALE),
        "w2": _tile_w(np.asarray(w2, np.float32), P,
                      ml_dtypes.float8_e4m3, F8_SCALE),
        "b1": np.asarray(b1, np.float32),
        "b2": np.asarray(b2, np.float32),
        "ln1_w": np.asarray(ln1_w, np.float32),
        "ln1_b": np.asarray(ln1_b, np.float32),
        "ln2_w": np.asarray(ln2_w, np.float32),
        "ln2_b": np.asarray(ln2_b, np.float32),
    }
    in_maps = []
    for c in range(8):
        b, q = divmod(c, 4)
        xT = np.ascontiguousarray(x[b].T)             # [D, NSEQ]
        # rotate so this core's own tokens are always columns [0, NT)
        xTr = np.ascontiguousarray(np.roll(xT, -q * NT, axis=1))
        in_maps.append({
            "xT": np.ascontiguousarray(xTr.astype(bf)),
            **shared,
        })
    return in_maps


def kernel(x, w_qkv, w_out, ln1_w, ln1_b, w1, b1, w2, b2, ln2_w, ln2_b):
    in_maps = make_in_maps(x, w_qkv, w_out, ln1_w, ln1_b, w1, b1, w2, b2,
                           ln2_w, ln2_b)
    nc = _get_nc()
    res = run_bass_kernel_spmd(nc, in_maps, list(range(8)))

    out = np.empty((B, NSEQ, D), np.float32)
    for c in range(8):
        b, q = divmod(c, 4)
        out[b, q * NT:(q + 1) * NT, :] = res.results[c]["yT"].T
    return out
